# revision 25
# baseline (speedup 1.0000x reference)
"""Multi-head attention (B=4, S=2048, D=768, H=12) on 8 TRN2 NeuronCores.

Sharding: core = (batch b, query-half). Each core computes Q for its 1024
query rows and full-sequence K/V for its batch (K/V projection duplicated
across the 2 cores sharing a batch -> zero collectives), then SDPA + o_proj
for its rows. Output rows are disjoint across cores.

v4 design notes:
 - host preprocessing: hidden states transposed to xT [768, S] with the
   core's OWN query half in columns 0..1023 (keys are permutation
   invariant, so each core sees its queries first); rope cos/sin tables
   built on host in the same column order. Q-side tables/activations are
   column slices of the K-side ones -> no separate q inputs.
 - projections run dc-outer so each weight/x chunk is consumed as its
   DMA lands; input DMAs spread over the 3 DMA queues by first-use time.
 - attention: per (head-pair, key-tile) the 4 score matmuls (K=64, row
   tile_position pairs, j-outer so cross-row-group pairs are adjacent
   and can pack) write ONE [128,2048] psum tile; a single 2048-wide
   ScalarE Exp evicts both heads (1.85us < PE ~1.9us per step, so
   ScalarE never limits); PV accumulates with the ones-column denominator
   trick (psum row 64).
 - norm: psum rowsum row -> sbuf, reshape to 128 lanes via DMA, exact
   reciprocal, back, gpsimd partition_broadcast, fused scale on eviction.
   (A direct [1,1024] reciprocal is 6.4us on one DVE lane - never again.)
"""

from contextlib import ExitStack

import numpy as np

import concourse.bass as bass
import concourse.bacc as bacc
import concourse.mybir as mybir
import concourse.tile as tile
from concourse.bass import ds, ts
from concourse.bass_utils import run_bass_kernel_spmd

F32 = mybir.dt.float32
BF16 = mybir.dt.bfloat16
I16 = mybir.dt.int16
AF = mybir.ActivationFunctionType
ALU = mybir.AluOpType

B, S, D, H = 4, 2048, 768, 12
HD = 64
SQ = 1024          # query rows per core
DC = D // 128      # 6 d-chunks
ST = S // 128      # 16 seq tiles of 128
VW = 128           # Vaug head stride (aligned window)
ROPE_BASE = 10000.0
N_CORES = 8

# Schraudolph exp constants (bf16 bit space), HW-validated:
# bits_i16 = round(128*log2(e)*0.125 * s + (128*127 - 7.5))
EXP_A = float(128.0 * np.log2(np.e) * 0.125)
EXP_B = float(128 * 127 - 7.5)


def build_nc():
    nc = bacc.Bacc("TRN2", target_bir_lowering=False, debug=False,
                   num_devices=N_CORES)

    xTd = nc.dram_tensor("xT", [D, S], BF16, kind="ExternalInput")
    cosRd = nc.dram_tensor("cosR", [128, S], BF16, kind="ExternalInput")
    sinSd = nc.dram_tensor("sinS", [128, S], BF16, kind="ExternalInput")
    wqT = nc.dram_tensor("wqT", [D, D], BF16, kind="ExternalInput")
    wkT = nc.dram_tensor("wkT", [D, D], BF16, kind="ExternalInput")
    wvT = nc.dram_tensor("wvT", [D, D], BF16, kind="ExternalInput")
    woT = nc.dram_tensor("woT", [D, D], BF16, kind="ExternalInput")
    bq = nc.dram_tensor("bq", [D, 1], F32, kind="ExternalInput")
    bk = nc.dram_tensor("bk", [D, 1], F32, kind="ExternalInput")
    bv = nc.dram_tensor("bv", [1, D], BF16, kind="ExternalInput")
    out = nc.dram_tensor("out", [SQ, D], F32, kind="ExternalOutput")

    with tile.TileContext(nc) as tc:
        _body(nc, tc, xTd, cosRd, sinSd, wqT, wkT, wvT, woT,
              bq, bk, bv, out)
    nc.compile()
    return nc


def _body(nc, tc, xTd, cosRd, sinSd, wqT, wkT, wvT, woT, bq, bk, bv, out):
  with ExitStack() as ctx:
    const = ctx.enter_context(tc.tile_pool(name="const", bufs=1))
    persist = ctx.enter_context(tc.tile_pool(name="persist", bufs=1))

    ones_row = const.tile([1, 128], BF16, tag="ones_row")
    nc.gpsimd.memset(ones_row[:], 1.0)

    QT = [persist.tile([128, SQ], BF16, tag=f"QT{e}", name=f"QT{e}")
          for e in range(DC)]
    KT = [persist.tile([128, S], BF16, tag=f"KT{e}", name=f"KT{e}")
          for e in range(DC)]
    Vaug = [persist.tile([128, H * VW], BF16, tag=f"Vaug{st}",
                         name=f"Vaug{st}") for st in range(ST)]
    attnT = [persist.tile([128, SQ], BF16, tag=f"attnT{e}", name=f"attnT{e}")
             for e in range(DC)]
    cosR = persist.tile([128, S], BF16, tag="cosR", name="cosR")
    sinS = persist.tile([128, S], BF16, tag="sinS", name="sinS")

    proj_ctx = ExitStack()
    qkv_w = proj_ctx.enter_context(tc.tile_pool(name="qkv_w", bufs=1))
    xtp = proj_ctx.enter_context(tc.tile_pool(name="xtp", bufs=1))
    shp = proj_ctx.enter_context(tc.tile_pool(name="shift", bufs=2))

    xT = [xtp.tile([128, S], BF16, tag=f"xT{dc}", name=f"xT{dc}")
          for dc in range(DC)]
    wq_sb = [qkv_w.tile([128, D], BF16, tag=f"w_q{dc}", name=f"w_q{dc}")
             for dc in range(DC)]
    wk_sb = [qkv_w.tile([128, D], BF16, tag=f"w_k{dc}", name=f"w_k{dc}")
             for dc in range(DC)]
    wv_sb = [qkv_w.tile([128, D], BF16, tag=f"w_v{dc}", name=f"w_v{dc}")
             for dc in range(DC)]
    bq_sb = [qkv_w.tile([128, 1], F32, tag=f"bq{e}", name=f"bq{e}")
             for e in range(DC)]
    bk_sb = [qkv_w.tile([128, 1], F32, tag=f"bk{e}", name=f"bk{e}")
             for e in range(DC)]
    bv_sb = qkv_w.tile([1, D], BF16, tag="bv", name="bv_sb")

    # ---- input DMAs on 3 queues, ordered by first-use time ----
    # sync:   wq -> wk -> wo ; gpsimd: xT 0,1,4,5 -> bv (+memsets, shifts)
    # scalar: bq/bk -> xT 2,3 -> cosR/sinS -> wv (then eviction ACTs)
    for dc in range(DC):
        nc.sync.dma_start(wq_sb[dc][:], wqT[ts(dc, 128), :])
    for e in range(DC):
        nc.scalar.dma_start(bq_sb[e][:], bq[ts(e, 128), :])
        nc.scalar.dma_start(bk_sb[e][:], bk[ts(e, 128), :])
    for dc in (0, 1):
        nc.gpsimd.dma_start(xT[dc][:], xTd[ts(dc, 128), :])
    for dc in (2, 3):
        nc.scalar.dma_start(xT[dc][:], xTd[ts(dc, 128), :])
    for dc in (4, 5):
        nc.gpsimd.dma_start(xT[dc][:], xTd[ts(dc, 128), :])
    for dc in range(DC):
        nc.sync.dma_start(wk_sb[dc][:], wkT[ts(dc, 128), :])
    nc.scalar.dma_start(cosR[:], cosRd[:])
    nc.scalar.dma_start(sinS[:], sinSd[:])
    nc.gpsimd.dma_start(bv_sb[:], bv[:])
    for dc in range(DC):
        nc.scalar.dma_start(wv_sb[dc][:], wvT[ts(dc, 128), :])

    # Vaug ones column + junk zero (gpsimd, off critical path)
    for st in range(ST):
        va = Vaug[st][:].rearrange("p (h x) -> p h x", x=VW)
        nc.gpsimd.memset(va[:, :, 64:65], 1.0)
        nc.gpsimd.memset(va[:, :, 65:VW], 0.0)

    # ---- projections (dc-outer: consume chunks as they land) ----
    with tc.tile_pool(name="proj_ps", bufs=1, space="PSUM") as pps:

        def proj_block(dst, w_sb, b_sb, j, xoff=0):
            ps = [pps.tile([128, 512], F32, tag=f"pp{e}", name=f"pp{e}")
                  for e in range(DC)]
            for dc in range(DC):
                for e in range(DC):
                    nc.tensor.matmul(ps[e][:], w_sb[dc][:, ts(e, 128)],
                                     xT[dc][:, ds(xoff + j * 512, 512)],
                                     start=(dc == 0), stop=(dc == DC - 1))
            for e in range(DC):
                nc.scalar.activation(dst[e][:, ts(j, 512)], ps[e][:],
                                     AF.Identity, bias=b_sb[e][:])

        def rope_inplace(dst_chunks, n_total, only=None):
            for e in (range(DC) if only is None else [only]):
                sh = shp.tile([128, n_total], BF16, tag="shift", name="sh")
                for q in range(4):
                    src_q = (q // 2) * 2 + (1 - q % 2)  # 0<->32, 64<->96
                    nc.gpsimd.dma_start(sh[ds(32 * q, 32), :],
                                        dst_chunks[e][ds(32 * src_q, 32), :])
                tmp = shp.tile([128, n_total], BF16, tag="ropetmp",
                               name="ropetmp")
                nc.vector.tensor_mul(tmp[:], sh[:], sinS[:, 0:n_total])
                nc.vector.tensor_mul(dst_chunks[e][:], dst_chunks[e][:],
                                     cosR[:, 0:n_total])
                nc.vector.tensor_add(dst_chunks[e][:], dst_chunks[e][:],
                                     tmp[:])

        def v_proj(st):
            for nt in range(2):
                p = pps.tile([128, 384], F32, tag=f"vp{nt}", name="vproj_p")
                for dc in range(DC):
                    nc.tensor.matmul(p[:], xT[dc][:, ts(st, 128)],
                                     wv_sb[dc][:, ts(nt, 384)],
                                     start=(dc == 0), stop=False)
                nc.tensor.matmul(p[:], ones_row[:], bv_sb[:, ts(nt, 384)],
                                 start=False, stop=True)
                dst = Vaug[st][:].rearrange("p (h x) -> p h x", x=VW)
                nc.scalar.activation(
                    dst[:, ds(nt * 6, 6), 0:64],
                    p.rearrange("p (h hd) -> p h hd", hd=64), AF.Identity)

        for j in range(SQ // 512):
            proj_block(QT, wq_sb, bq_sb, j)
        rope_inplace(QT, SQ)
        for j in range(S // 512):
            proj_block(KT, wk_sb, bk_sb, j)
        for e in range(DC):
            rope_inplace(KT, S, only=e)
        for st in range(ST):
            v_proj(st)
    proj_ctx.close()

    # ---- o_proj weights (DMA during attention) ----
    wop = ctx.enter_context(tc.tile_pool(name="wop", bufs=1))
    wo_sb = [wop.tile([128, D], BF16, tag=f"w_o{dc}", name=f"w_o{dc}")
             for dc in range(DC)]
    for dc in range(DC):
        nc.sync.dma_start(wo_sb[dc][:], woT[ts(dc, 128), :])

    # ---- attention ----
    # PV lags LAG steps behind scores and the lag extends ACROSS head-pair
    # boundaries: while pair hp's last PV accumulations drain, pair hp+1's
    # scores already stream, so the PE never idles long enough for the HAM
    # clock throttle to re-engage at boundaries.
    with (tc.tile_pool(name="scores_ps", bufs=1, space="PSUM") as sps,
          tc.tile_pool(name="pv_ps", bufs=2, space="PSUM") as pvps,
          tc.tile_pool(name="expp", bufs=8) as expp,
          tc.tile_pool(name="attn_sb", bufs=3) as asb):
        LAG = 3
        pvq = []          # queued (hp, skt, pv tiles, ex pair) for PV
        pv_of = {}

        def do_pv(hp, skt, pv, exs):
            for i in range(2):
                h = 2 * hp + i
                for j in range(SQ // 512):
                    nc.tensor.matmul(
                        pv[i][:, ts(j, 512)],
                        Vaug[skt][:, ds(h * VW, 128)],
                        exs[i][:, ts(j, 512)],
                        start=(skt == 0), stop=(skt == ST - 1))
            if skt == ST - 1:
                norm_pair(hp, pv)

        def norm_pair(hp, pv):
            rsrow = [asb.tile([1, SQ], F32, tag=f"rsrow{i}", name="rsrow")
                     for i in range(2)]
            c8 = [asb.tile([128, SQ // 128], F32, tag=f"c8{i}", name="c8")
                  for i in range(2)]
            r8 = [asb.tile([128, SQ // 128], F32, tag=f"r8{i}", name="r8")
                  for i in range(2)]
            recb = [asb.tile([1, SQ], F32, tag=f"recb{i}", name="recb")
                    for i in range(2)]
            rbs = [asb.tile([64, SQ], F32, tag=f"rbs{i}", name="rbs")
                   for i in range(2)]
            for i in range(2):
                nc.vector.tensor_copy(rsrow[i][:], pv[i][ds(64, 1), :])
            for i in range(2):
                nc.gpsimd.dma_start(c8[i][:], rsrow[i][:])
            for i in range(2):
                nc.vector.reciprocal(r8[i][:], c8[i][:])
            for i in range(2):
                nc.gpsimd.dma_start(recb[i][:], r8[i][:])
            for i in range(2):
                nc.gpsimd.partition_broadcast(rbs[i][:], recb[i][:])
            for i in range(2):
                nc.vector.tensor_mul(attnT[hp][ds(64 * i, 64), :],
                                     pv[i][ds(0, 64), :], rbs[i][:])

        for hp in range(DC):          # head pair = e-chunk
            pv_of[hp] = [pvps.tile([128, SQ], F32, tag="pv", name=f"pv{i}")
                         for i in range(2)]
            for skt in range(ST):
                sc = [sps.tile([128, SQ], F32, tag=f"sc{i}", name=f"sc{i}")
                      for i in range(2)]
                # j-outer: (i0,j) and (i1,j) adjacent -> row-group packing
                for j in range(SQ // 512):
                    for i in range(2):
                        nc.tensor.matmul(
                            sc[i][:, ts(j, 512)],
                            KT[hp][ds(64 * i, 64), ts(skt, 128)],
                            QT[hp][ds(64 * i, 64), ts(j, 512)],
                            start=True, stop=True,
                            tile_position=(64 * i, 0))
                exs = []
                for i in range(2):
                    e = expp.tile([128, SQ], BF16, tag="exp", name="expt")
                    # hp 0: all exps on ScalarE -- the DVE FIFO is still
                    # draining rope-K there, and a TS queued behind it
                    # stalls score-psum recycling
                    if i == 1 and skt % 2 == 1 and hp > 0:
                        nc.vector.tensor_scalar(e[:].bitcast(I16), sc[i][:],
                                                EXP_A, EXP_B,
                                                ALU.mult, ALU.add)
                    else:
                        nc.scalar.activation(e[:], sc[i][:], AF.Exp,
                                             scale=0.125)
                    exs.append(e)
                pvq.append((hp, skt, pv_of[hp], exs))
                if len(pvq) > LAG:
                    do_pv(*pvq.pop(0))
        while pvq:
            do_pv(*pvq.pop(0))

    # ---- o_proj (row-major out) ----
    # waves of 4 psum tiles with dc-planes interleaved: the dc<5 matmuls
    # of a wave run while the last pair's norm (gating attnT[5]) drains,
    # instead of a single dc=5-blocked matmul stalling the PE FIFO
    with (tc.tile_pool(name="o_ps", bufs=2, space="PSUM") as ops,
          tc.tile_pool(name="o_sb", bufs=4) as osb):
        tiles = [(st, nt) for st in range(SQ // 128) for nt in range(2)]
        for w0 in range(0, len(tiles), 4):
            wave = tiles[w0:w0 + 4]
            ps = [ops.tile([128, 384], F32, tag=f"o{k}", name="o_p")
                  for k in range(len(wave))]
            for dc in range(DC):
                for k, (st, nt) in enumerate(wave):
                    nc.tensor.matmul(ps[k][:], attnT[dc][:, ts(st, 128)],
                                     wo_sb[dc][:, ts(nt, 384)],
                                     start=(dc == 0), stop=(dc == DC - 1))
            for k, (st, nt) in enumerate(wave):
                o = osb.tile([128, 384], F32, tag="o_out", name="o_out")
                nc.scalar.activation(o[:], ps[k][:], AF.Identity)
                nc.sync.dma_start(out[ts(st, 128), ts(nt, 384)], o[:])


_NC_CACHE = None


def _get_nc():
    global _NC_CACHE
    if _NC_CACHE is None:
        _NC_CACHE = build_nc()
    return _NC_CACHE


def _host_rope_tables(pos_row):
    """cosR/sinS [128, n] f64->bf16 from a position row [n] (int32)."""
    import ml_dtypes
    bf16 = ml_dtypes.bfloat16
    invf = (1.0 / ROPE_BASE) ** (np.arange(32, dtype=np.float64) / 32.0)
    freqs = pos_row.astype(np.float64)[None, :] * invf[:, None]  # [32, n]
    cos32 = np.cos(freqs)
    sin32 = np.sin(freqs)
    cos64 = np.concatenate([cos32, cos32], axis=0)            # [64, n]
    sin64 = np.concatenate([-sin32, sin32], axis=0)           # [64, n]
    cosR = np.concatenate([cos64, cos64], axis=0)             # [128, n]
    sinS = np.concatenate([sin64, sin64], axis=0)
    return (np.ascontiguousarray(cosR.astype(np.float32)).astype(bf16),
            np.ascontiguousarray(sinS.astype(np.float32)).astype(bf16))


def kernel(hidden_states, position_ids, wq, bq, wk, bk, wv, bv, wo,
           _trace=False):
    import ml_dtypes
    bf16 = ml_dtypes.bfloat16
    hidden_states = np.asarray(hidden_states, dtype=np.float32)
    position_ids = np.asarray(position_ids, dtype=np.int32)
    wqT = np.ascontiguousarray(np.asarray(wq, np.float32).T.astype(bf16))
    wkT = np.ascontiguousarray(np.asarray(wk, np.float32).T.astype(bf16))
    wvT = np.ascontiguousarray(np.asarray(wv, np.float32).T.astype(bf16))
    woT = np.ascontiguousarray(np.asarray(wo, np.float32).T.astype(bf16))
    bq_c = np.ascontiguousarray(np.asarray(bq, np.float32).reshape(D, 1))
    bk_c = np.ascontiguousarray(np.asarray(bk, np.float32).reshape(D, 1))
    bv_r = np.ascontiguousarray(
        np.asarray(bv, np.float32).reshape(1, D).astype(bf16))

    nc = _get_nc()
    in_maps = []
    for core in range(N_CORES):
        b, half = core // 2, core % 2
        # permute the sequence so this core's query half is first; keys
        # are order-invariant (K/V/rope tables permuted consistently)
        perm = (np.r_[SQ:2 * SQ, 0:SQ] if half else np.arange(S))
        hsT = np.ascontiguousarray(
            hidden_states[b].T[:, perm].astype(bf16))
        cosR, sinS = _host_rope_tables(position_ids[b][perm])
        in_maps.append({
            "xT": hsT, "cosR": cosR, "sinS": sinS,
            "wqT": wqT, "wkT": wkT, "wvT": wvT, "woT": woT,
            "bq": bq_c, "bk": bk_c, "bv": bv_r,
        })
    res = run_bass_kernel_spmd(nc, in_maps, list(range(N_CORES)),
                               trace=_trace)
    outp = np.empty((B, S, D), np.float32)
    for core in range(N_CORES):
        b, half = core // 2, core % 2
        outp[b, half * SQ:(half + 1) * SQ] = res.results[core]["out"]
    if _trace:
        kernel._last_exec_time_ns = res.exec_time_ns
        kernel._last_results = res
    return outp
